# revision 12
# baseline (speedup 1.0000x reference)
"""CrossProductLayer kernel for Trainium2 (Bass/Tile), 8-core data parallel.

out[b, :] = concat(x[b]**2, x[b], 0.5 * x[b,i]*x[b,j] for i<j) * w

Full inputs:  x [16384, 128] f32, w [8384] f32.
Full output:  [16384, 8384] f32.

Sharding: pure data parallel on the batch dim — each of the 8 cores gets
2048 rows of x; small constants (w broadcast, gather matrices) are
replicated. No collectives (forward only).

Strategy: TensorE manufactures both pair operands with bf16 matmuls
against constant one-hot gather matrices (w folded into the J side):

  A'[p, f] = x[p, I(f)]            = xT_bf16.T @ SA   (PE -> PSUM)
  B'[p, f] = x[p, J(f)] * 0.5*w[f] = xT_bf16.T @ SBW  (PE -> PSUM)

The BIR verifier allows at most one PSUM operand per DVE op and GpSimd
none, so ScalarE (otherwise idle) copies B' PSUM->SBUF and DVE does one
wide TT per 1024-column chunk: out = A'(psum) * B'(sbuf). GpSimd takes
the squares/singles blocks. All x transposes run in a prologue so the
PE streams matmuls back-to-back (keeps its p-state clock high).

Per-core structure (16 row-tiles of 128 batch rows):
  prologue: load x tiles, PE-transpose each, ScalarE-cast to bf16
  per tile: 8 chunks x (2+2 PE matmuls, ScalarE copy, DVE TT)
            + GpSimd squares/singles; 4 column-chunked stores
"""

import numpy as np

B = 16384
NI = 128
NPAIRS = (NI * (NI - 1)) // 2  # 8128
NF = 2 * NI + NPAIRS  # 8384
NCORES = 8
ROWS = B // NCORES  # 2048
TILE_P = 128
TILES = ROWS // TILE_P  # 16
PAIRS_OFF = 2 * NI  # 256
MM = 512  # matmul chunk (one PSUM bank)
CHUNK = 1024  # combine chunk (two PSUM banks)
NCHUNK = (NPAIRS + CHUNK - 1) // CHUNK  # 8 (last chunk 960 wide)

_CACHE = {}


def _pair_block_start(i):
    """Start offset (pairs-only indexing) of pair block i (pairs (i, j>i))."""
    return i * 127 - i * (i - 1) // 2


def _row_window(lo, hi):
    """Smallest HW-aligned lhsT partition window [w0, w1) covering rows
    [lo, hi]. Matmul tile_position allows starts {0,32,64,96} for K<=32,
    {0,64} for K<=64, else the full 128 rows."""
    for sz, starts in ((32, (0, 32, 64, 96)), (64, (0, 64))):
        for st in starts:
            if st <= lo and hi < st + sz:
                return st, st + sz
    return 0, 128


def _chunk_windows():
    """Per pair-chunk (A-side, B-side) lhsT/rhs row windows. The one-hot
    gather matrices are zero outside these rows, so trimming K only drops
    zero contributions but shrinks each LDWEIGHTS."""
    wins = []
    for c in range(NCHUNK):
        c0 = c * CHUNK
        c1 = min(c0 + CHUNK, NPAIRS)
        i_lo = max(i for i in range(NI - 1) if _pair_block_start(i) <= c0)
        i_hi = max(i for i in range(NI - 1) if _pair_block_start(i) < c1)
        wins.append((_row_window(i_lo, i_hi), _row_window(i_lo + 1, NI - 1)))
    return wins


def _build_nc():
    import os

    os.environ["TILE_EXHAUSTIVE_MEMORY_SHARE_CHECK"] = "1"
    from concourse import bacc
    import concourse.mybir as mybir
    from concourse.tile import TileContext

    f32 = mybir.dt.float32
    bf16 = mybir.dt.bfloat16
    mult = mybir.AluOpType.mult
    nc = bacc.Bacc(
        "TRN2",
        target_bir_lowering=False,
        debug=False,
        num_devices=NCORES,
    )
    x_d = nc.dram_tensor("x", [ROWS, NI], f32, kind="ExternalInput")
    w2_d = nc.dram_tensor("w2", [NI, 2 * NI], f32, kind="ExternalInput")
    sa_d = nc.dram_tensor("sa", [NI, NPAIRS], bf16, kind="ExternalInput")
    sbw_d = nc.dram_tensor("sbw", [NI, NPAIRS], bf16, kind="ExternalInput")
    id_d = nc.dram_tensor("ident", [NI, NI], f32, kind="ExternalInput")
    o_d = nc.dram_tensor("out", [ROWS, NF], f32, kind="ExternalOutput")

    wins = _chunk_windows()
    with TileContext(nc) as tc:
        with (
            tc.tile_pool(name="const", bufs=1) as cp,
            tc.tile_pool(name="sqp", bufs=2) as sqp,
            tc.tile_pool(name="bwp", bufs=3) as bwp,
            tc.tile_pool(name="op", bufs=4) as op,
            tc.tile_pool(name="pa", bufs=2, space="PSUM") as pap,
            tc.tile_pool(name="pb", bufs=2, space="PSUM") as pbp,
        ):
            w2_t = cp.tile([NI, 2 * NI], f32, name="w2")
            id_t = cp.tile([NI, NI], f32, name="ident")
            x_all = cp.tile([NI, ROWS], f32, name="x_all")
            xT_all = cp.tile([NI, ROWS], bf16, name="xT_all")
            sa_t = cp.tile([NI, NPAIRS], bf16, name="sa")
            sbw_t = cp.tile([NI, NPAIRS], bf16, name="sbw")
            # x + ident load first: the in-order PE must run all 16
            # transposes before the first gather matmul, so their inputs
            # must not queue behind the 4 MB gather-matrix loads
            nc.sync.dma_start(out=id_t[:], in_=id_d[:])
            for t in range(TILES):
                r0 = t * TILE_P
                nc.sync.dma_start(
                    out=x_all[:, r0 : r0 + TILE_P], in_=x_d[r0 : r0 + TILE_P]
                )
            for t in range(TILES):
                c0 = t * TILE_P
                tp = pap.tile([TILE_P, CHUNK], f32, name="pa")
                nc.tensor.transpose(
                    tp[:, 0:TILE_P], x_all[:, c0 : c0 + TILE_P], id_t[:]
                )
                nc.scalar.copy(
                    out=xT_all[:, c0 : c0 + TILE_P], in_=tp[:, 0:TILE_P]
                )
            nc.sync.dma_start(out=w2_t[:], in_=w2_d[:])
            # gather-matrix loads, quarter-column chunks for early unblock
            QL = NPAIRS // 4
            for q in range(4):
                q1 = NPAIRS if q == 3 else (q + 1) * QL
                nc.sync.dma_start(out=sa_t[:, q * QL : q1], in_=sa_d[:, q * QL : q1])
                nc.sync.dma_start(
                    out=sbw_t[:, q * QL : q1], in_=sbw_d[:, q * QL : q1]
                )

            for t in range(TILES):
                r0 = t * TILE_P
                xs = x_all[:, r0 : r0 + TILE_P]
                xT_bf = xT_all[:, r0 : r0 + TILE_P]
                o_t = op.tile([TILE_P, NF], f32, name="o_t")
                # squares + singles on GpSimd (SBUF-only engine)
                s_t = sqp.tile([TILE_P, NI], f32, name="s_t")
                nc.gpsimd.tensor_tensor(out=s_t[:], in0=xs, in1=xs, op=mult)
                nc.gpsimd.tensor_tensor(
                    out=o_t[:, 0:NI], in0=s_t[:], in1=w2_t[:, 0:NI], op=mult
                )
                nc.gpsimd.tensor_tensor(
                    out=o_t[:, NI : 2 * NI],
                    in0=xs,
                    in1=w2_t[:, NI : 2 * NI],
                    op=mult,
                )
                # pair chunks: 2+2 matmuls, ScalarE psum->sbuf, DVE combine
                for c in range(NCHUNK):
                    c0 = c * CHUNK
                    cw = min(CHUNK, NPAIRS - c0)
                    (a0, a1), (b0, b1) = wins[c]
                    pa = pap.tile([TILE_P, CHUNK], f32, name="pa")
                    pb = pbp.tile([TILE_P, CHUNK], f32, name="pb")
                    for h in (0, MM):
                        hw = min(MM, cw - h)
                        nc.tensor.matmul(
                            pa[:, h : h + hw],
                            lhsT=xT_all[a0:a1, r0 : r0 + TILE_P],
                            rhs=sa_t[a0:a1, c0 + h : c0 + h + hw],
                        )
                        nc.tensor.matmul(
                            pb[:, h : h + hw],
                            lhsT=xT_all[b0:b1, r0 : r0 + TILE_P],
                            rhs=sbw_t[b0:b1, c0 + h : c0 + h + hw],
                        )
                    bw = bwp.tile([TILE_P, CHUNK], f32, name="bw")
                    nc.scalar.copy(out=bw[:, 0:cw], in_=pb[:, 0:cw])
                    nc.vector.tensor_tensor(
                        out=o_t[:, PAIRS_OFF + c0 : PAIRS_OFF + c0 + cw],
                        in0=pa[:, 0:cw],
                        in1=bw[:, 0:cw],
                        op=mult,
                    )
                # store in 8 chunk-aligned column slices (finer grain keeps
                # the DMA queues evenly fed and shrinks the tail drain)
                bounds = [0] + [PAIRS_OFF + c * CHUNK for c in range(1, NCHUNK)] + [NF]
                for q in range(len(bounds) - 1):
                    nc.sync.dma_start(
                        out=o_d[r0 : r0 + TILE_P, bounds[q] : bounds[q + 1]],
                        in_=o_t[:, bounds[q] : bounds[q + 1]],
                    )
    nc.compile()
    return nc


def _get_nc():
    if "nc" not in _CACHE:
        _CACHE["nc"] = _build_nc()
    return _CACHE["nc"]


def _prep_in_maps(x, w):
    import ml_dtypes

    bf16 = ml_dtypes.bfloat16
    x = np.ascontiguousarray(np.asarray(x, dtype=np.float32))
    w = np.asarray(w, dtype=np.float32)

    ii, jj = np.triu_indices(NI, k=1)
    cols = np.arange(NPAIRS)
    sa = np.zeros((NI, NPAIRS), dtype=np.float32)
    sa[ii, cols] = 1.0
    sbw = np.zeros((NI, NPAIRS), dtype=np.float32)
    sbw[jj, cols] = 0.5 * w[PAIRS_OFF:]
    sa = np.ascontiguousarray(sa.astype(bf16))
    sbw = np.ascontiguousarray(sbw.astype(bf16))
    w2 = np.ascontiguousarray(
        np.broadcast_to(w[None, : 2 * NI], (NI, 2 * NI)).astype(np.float32)
    )
    ident = np.eye(NI, dtype=np.float32)

    return [
        {
            "x": np.ascontiguousarray(x[c * ROWS : (c + 1) * ROWS]),
            "w2": w2,
            "sa": sa,
            "sbw": sbw,
            "ident": ident,
        }
        for c in range(NCORES)
    ]


def _run(x, w, trace=False, tmpdir=None):
    from concourse.bass_utils import run_bass_kernel_spmd

    nc = _get_nc()
    in_maps = _prep_in_maps(x, w)
    res = run_bass_kernel_spmd(
        nc, in_maps, list(range(NCORES)), trace=trace, tmpdir=tmpdir
    )
    out = np.concatenate([res.results[c]["out"] for c in range(NCORES)], axis=0)
    return out, res


def kernel(**inputs):
    out, _ = _run(inputs["x"], inputs["w"])
    return out


# revision 13
# speedup vs baseline: 1.3694x; 1.3694x over previous
"""CrossProductLayer kernel for Trainium2 (Bass/Tile), 8-core data parallel.

out[b, :] = concat(x[b]**2, x[b], 0.5 * x[b,i]*x[b,j] for i<j) * w

Full inputs:  x [16384, 128] f32, w [8384] f32.
Full output:  [16384, 8384] f32.

Sharding: pure data parallel on the batch dim — each of the 8 cores gets
2048 rows of x; small constants (w broadcast, gather matrices) are
replicated. No collectives (forward only).

Strategy: TensorE manufactures both pair operands with bf16 matmuls
against constant one-hot gather matrices (w folded into the J side):

  A'[p, f] = x[p, I(f)]            = xT_bf16.T @ SA   (PE -> PSUM)
  B'[p, f] = x[p, J(f)] * 0.5*w[f] = xT_bf16.T @ SBW  (PE -> PSUM)

The BIR verifier allows at most one PSUM operand per DVE op and GpSimd
none, so ScalarE (otherwise idle) copies B' PSUM->SBUF and DVE does one
wide TT per 1024-column chunk: out = A'(psum) * B'(sbuf). GpSimd takes
the squares/singles blocks. All x transposes run in a prologue so the
PE streams matmuls back-to-back (keeps its p-state clock high).

Per-core structure (16 row-tiles of 128 batch rows):
  prologue: load x tiles, PE-transpose each, ScalarE-cast to bf16
  per tile: 8 chunks x (2+2 PE matmuls, ScalarE copy, DVE TT)
            + GpSimd squares/singles; 4 column-chunked stores
"""

import numpy as np

B = 16384
NI = 128
NPAIRS = (NI * (NI - 1)) // 2  # 8128
NF = 2 * NI + NPAIRS  # 8384
NCORES = 8
ROWS = B // NCORES  # 2048
TILE_P = 128
TILES = ROWS // TILE_P  # 16
PAIRS_OFF = 2 * NI  # 256
MM = 512  # matmul chunk (one PSUM bank)
CHUNK = 1024  # combine chunk (two PSUM banks)
NCHUNK = (NPAIRS + CHUNK - 1) // CHUNK  # 8 (last chunk 960 wide)

_CACHE = {}


def _pair_block_start(i):
    """Start offset (pairs-only indexing) of pair block i (pairs (i, j>i))."""
    return i * 127 - i * (i - 1) // 2


def _row_window(lo, hi):
    """Smallest HW-aligned lhsT partition window [w0, w1) covering rows
    [lo, hi]. Matmul tile_position allows starts {0,32,64,96} for K<=32,
    {0,64} for K<=64, else the full 128 rows."""
    for sz, starts in ((32, (0, 32, 64, 96)), (64, (0, 64))):
        for st in starts:
            if st <= lo and hi < st + sz:
                return st, st + sz
    return 0, 128


def _chunk_windows():
    """Per pair-chunk (A-side, B-side) lhsT/rhs row windows. The one-hot
    gather matrices are zero outside these rows, so trimming K only drops
    zero contributions but shrinks each LDWEIGHTS."""
    wins = []
    for c in range(NCHUNK):
        c0 = c * CHUNK
        c1 = min(c0 + CHUNK, NPAIRS)
        i_lo = max(i for i in range(NI - 1) if _pair_block_start(i) <= c0)
        i_hi = max(i for i in range(NI - 1) if _pair_block_start(i) < c1)
        wins.append((_row_window(i_lo, i_hi), _row_window(i_lo + 1, NI - 1)))
    return wins


def _build_nc():
    import os

    os.environ["TILE_EXHAUSTIVE_MEMORY_SHARE_CHECK"] = "1"
    from concourse import bacc
    import concourse.mybir as mybir
    from concourse.tile import TileContext

    f32 = mybir.dt.float32
    bf16 = mybir.dt.bfloat16
    mult = mybir.AluOpType.mult
    nc = bacc.Bacc(
        "TRN2",
        target_bir_lowering=False,
        debug=False,
        num_devices=NCORES,
    )
    x_d = nc.dram_tensor("x", [ROWS, NI], f32, kind="ExternalInput")
    w2_d = nc.dram_tensor("w2", [NI, 2 * NI], f32, kind="ExternalInput")
    sa_d = nc.dram_tensor("sa", [NI, NPAIRS], bf16, kind="ExternalInput")
    sbw_d = nc.dram_tensor("sbw", [NI, NPAIRS], bf16, kind="ExternalInput")
    id_d = nc.dram_tensor("ident", [NI, NI], f32, kind="ExternalInput")
    o_d = nc.dram_tensor("out", [ROWS, NF], f32, kind="ExternalOutput")

    wins = _chunk_windows()
    with TileContext(nc) as tc:
        with (
            tc.tile_pool(name="const", bufs=1) as cp,
            tc.tile_pool(name="sqp", bufs=2) as sqp,
            tc.tile_pool(name="bwp", bufs=3) as bwp,
            tc.tile_pool(name="op", bufs=4) as op,
            tc.tile_pool(name="pa", bufs=2, space="PSUM") as pap,
            tc.tile_pool(name="pb", bufs=2, space="PSUM") as pbp,
        ):
            w2_t = cp.tile([NI, 2 * NI], f32, name="w2")
            id_t = cp.tile([NI, NI], f32, name="ident")
            x_all = cp.tile([NI, ROWS], f32, name="x_all")
            xT_all = cp.tile([NI, ROWS], bf16, name="xT_all")
            sa_t = cp.tile([NI, NPAIRS], bf16, name="sa")
            sbw_t = cp.tile([NI, NPAIRS], bf16, name="sbw")
            # x + ident load first: the in-order PE must run all 16
            # transposes before the first gather matmul, so their inputs
            # must not queue behind the 4 MB gather-matrix loads
            nc.sync.dma_start(out=id_t[:], in_=id_d[:])
            for t in range(TILES):
                r0 = t * TILE_P
                nc.sync.dma_start(
                    out=x_all[:, r0 : r0 + TILE_P], in_=x_d[r0 : r0 + TILE_P]
                )
            for t in range(TILES):
                c0 = t * TILE_P
                tp = pap.tile([TILE_P, CHUNK], f32, name="pa")
                nc.tensor.transpose(
                    tp[:, 0:TILE_P], x_all[:, c0 : c0 + TILE_P], id_t[:]
                )
                nc.scalar.copy(
                    out=xT_all[:, c0 : c0 + TILE_P], in_=tp[:, 0:TILE_P]
                )
            nc.sync.dma_start(out=w2_t[:], in_=w2_d[:])
            # gather-matrix loads, quarter-column chunks for early unblock
            QL = NPAIRS // 4
            for q in range(4):
                q1 = NPAIRS if q == 3 else (q + 1) * QL
                nc.sync.dma_start(out=sa_t[:, q * QL : q1], in_=sa_d[:, q * QL : q1])
                nc.sync.dma_start(
                    out=sbw_t[:, q * QL : q1], in_=sbw_d[:, q * QL : q1]
                )

            for t in range(TILES):
                r0 = t * TILE_P
                xs = x_all[:, r0 : r0 + TILE_P]
                xT_bf = xT_all[:, r0 : r0 + TILE_P]
                o_t = op.tile([TILE_P, NF], f32, name="o_t")
                # squares + singles on GpSimd (SBUF-only engine)
                s_t = sqp.tile([TILE_P, NI], f32, name="s_t")
                nc.gpsimd.tensor_tensor(out=s_t[:], in0=xs, in1=xs, op=mult)
                nc.gpsimd.tensor_tensor(
                    out=o_t[:, 0:NI], in0=s_t[:], in1=w2_t[:, 0:NI], op=mult
                )
                nc.gpsimd.tensor_tensor(
                    out=o_t[:, NI : 2 * NI],
                    in0=xs,
                    in1=w2_t[:, NI : 2 * NI],
                    op=mult,
                )
                # pair chunks: 2+2 matmuls, ScalarE psum->sbuf, DVE combine
                for c in range(NCHUNK):
                    c0 = c * CHUNK
                    cw = min(CHUNK, NPAIRS - c0)
                    pa = pap.tile([TILE_P, CHUNK], f32, name="pa")
                    pb = pbp.tile([TILE_P, CHUNK], f32, name="pb")
                    for h in (0, MM):
                        hw = min(MM, cw - h)
                        nc.tensor.matmul(
                            pa[:, h : h + hw],
                            lhsT=xT_bf,
                            rhs=sa_t[:, c0 + h : c0 + h + hw],
                        )
                        nc.tensor.matmul(
                            pb[:, h : h + hw],
                            lhsT=xT_bf,
                            rhs=sbw_t[:, c0 + h : c0 + h + hw],
                        )
                    bw = bwp.tile([TILE_P, CHUNK], f32, name="bw")
                    nc.scalar.copy(out=bw[:, 0:cw], in_=pb[:, 0:cw])
                    nc.vector.tensor_tensor(
                        out=o_t[:, PAIRS_OFF + c0 : PAIRS_OFF + c0 + cw],
                        in0=pa[:, 0:cw],
                        in1=bw[:, 0:cw],
                        op=mult,
                    )
                # store in 8 chunk-aligned column slices (finer grain keeps
                # the DMA queues evenly fed and shrinks the tail drain)
                bounds = [0] + [PAIRS_OFF + c * CHUNK for c in range(1, NCHUNK)] + [NF]
                for q in range(len(bounds) - 1):
                    nc.sync.dma_start(
                        out=o_d[r0 : r0 + TILE_P, bounds[q] : bounds[q + 1]],
                        in_=o_t[:, bounds[q] : bounds[q + 1]],
                    )
    nc.compile()
    return nc


def _get_nc():
    if "nc" not in _CACHE:
        _CACHE["nc"] = _build_nc()
    return _CACHE["nc"]


def _prep_in_maps(x, w):
    import ml_dtypes

    bf16 = ml_dtypes.bfloat16
    x = np.ascontiguousarray(np.asarray(x, dtype=np.float32))
    w = np.asarray(w, dtype=np.float32)

    ii, jj = np.triu_indices(NI, k=1)
    cols = np.arange(NPAIRS)
    sa = np.zeros((NI, NPAIRS), dtype=np.float32)
    sa[ii, cols] = 1.0
    sbw = np.zeros((NI, NPAIRS), dtype=np.float32)
    sbw[jj, cols] = 0.5 * w[PAIRS_OFF:]
    sa = np.ascontiguousarray(sa.astype(bf16))
    sbw = np.ascontiguousarray(sbw.astype(bf16))
    w2 = np.ascontiguousarray(
        np.broadcast_to(w[None, : 2 * NI], (NI, 2 * NI)).astype(np.float32)
    )
    ident = np.eye(NI, dtype=np.float32)

    return [
        {
            "x": np.ascontiguousarray(x[c * ROWS : (c + 1) * ROWS]),
            "w2": w2,
            "sa": sa,
            "sbw": sbw,
            "ident": ident,
        }
        for c in range(NCORES)
    ]


def _run(x, w, trace=False, tmpdir=None):
    from concourse.bass_utils import run_bass_kernel_spmd

    nc = _get_nc()
    in_maps = _prep_in_maps(x, w)
    res = run_bass_kernel_spmd(
        nc, in_maps, list(range(NCORES)), trace=trace, tmpdir=tmpdir
    )
    out = np.concatenate([res.results[c]["out"] for c in range(NCORES)], axis=0)
    return out, res


def kernel(**inputs):
    out, _ = _run(inputs["x"], inputs["w"])
    return out
